# revision 3
# baseline (speedup 1.0000x reference)
"""KIVI attention wrapper — Trainium2 Bass kernel v4: fully local sharding.

Core c handles batch bc=c//2 and query-half hc=c%2 (512 query tokens),
for ALL 16 heads, in 8 head-group passes (2 heads each). K/V for the
batch's full 1024 positions are computed locally (duplicated across the
2 cores sharing a batch: +27us PE) — in exchange there are NO
collectives and no cross-core dependencies at all. c_proj is one local
N=512 pass over the 8 head-group outputs at the end; the host
concatenates the per-core 512-token output slices.

Per-core engine budget (cost model): PE ~150us, DVE ~110us, ACT ~95us,
POOL ~40us, DMA ~20MB.
"""
import sys
sys.path.insert(0, '/opt/trn_rl_repo')
import numpy as np

P = 128
TOK = 4096
E = 1024
NB = 8
CH = 512
MAGIC = 8388608.0

_CACHE = {}


def _build(sim_single=False):
    import concourse.bacc as bacc
    import concourse.mybir as mybir
    import concourse.tile as tile

    f32 = mybir.dt.float32
    fmm = mybir.dt.float32r
    bf16 = mybir.dt.bfloat16
    X = mybir.AxisListType.X
    ADD = mybir.AluOpType.add
    MULT = mybir.AluOpType.mult
    MAX = mybir.AluOpType.max
    SUB = mybir.AluOpType.subtract
    EXP = mybir.ActivationFunctionType.Exp

    nc = bacc.Bacc("TRN2", target_bir_lowering=False, debug=False,
                   num_devices=(1 if sim_single else 8))

    # xt: this batch's X^T columns. wqkv3: [E, hg, 384] (q|k|v cols for the
    # 2 heads of group hg). m8t: this batch's mask/8 per kpos block.
    xt_ap = nc.dram_tensor("xt", [E, 1024], fmm, kind="ExternalInput").ap()
    w3_ap = nc.dram_tensor("wqkv3", [E, NB, 384], fmm, kind="ExternalInput").ap()
    wp_ap = nc.dram_tensor("wp", [E, E], bf16, kind="ExternalInput").ap()
    bqkv_ap = nc.dram_tensor("bqkv", [P, NB, 3], f32, kind="ExternalInput").ap()
    bp_ap = nc.dram_tensor("bp", [P, NB], f32, kind="ExternalInput").ap()
    m8t_ap = nc.dram_tensor("m8t", [P, NB], f32, kind="ExternalInput").ap()
    ident_ap = nc.dram_tensor("ident", [P, P], f32, kind="ExternalInput").ap()
    ones64_ap = nc.dram_tensor("ones64", [1, 64], fmm, kind="ExternalInput").ap()
    hc_ap = nc.dram_tensor("hcsel", [1, 1], mybir.dt.int32,
                           kind="ExternalInput").ap()  # unused on device
    yt_ap = nc.dram_tensor("yt", [E, CH], f32, kind="ExternalOutput").ap()

    with tile.TileContext(nc) as tc:
        with tc.tile_pool(name="const", bufs=1) as constp, \
             tc.tile_pool(name="xte", bufs=1) as xtp, \
             tc.tile_pool(name="whg", bufs=2) as whgp, \
             tc.tile_pool(name="big", bufs=2) as bigp, \
             tc.tile_pool(name="vn", bufs=2) as vnp, \
             tc.tile_pool(name="es", bufs=2) as esp, \
             tc.tile_pool(name="wk2", bufs=2) as wkp, \
             tc.tile_pool(name="otp", bufs=1) as otp, \
             tc.tile_pool(name="qkvps", bufs=2, space="PSUM") as mmps, \
             tc.tile_pool(name="scps", bufs=2, space="PSUM") as scps, \
             tc.tile_pool(name="avps", bufs=2, space="PSUM") as avps, \
             tc.tile_pool(name="smallps", bufs=2, space="PSUM") as smps:

            # ---------------- constants / weights ----------------
            identt = constp.tile([P, P], f32)
            nc.scalar.dma_start(identt[:], ident_ap)
            ones64 = constp.tile([1, 64], fmm)
            nc.scalar.dma_start(ones64[:], ones64_ap)
            onescol = constp.tile([P, 1], f32)
            nc.gpsimd.memset(onescol[:], 1.0)
            m8tt = constp.tile([P, NB], f32)
            nc.scalar.dma_start(m8tt[:], m8t_ap)
            bqkvt = constp.tile([P, NB * 3], f32)
            nc.scalar.dma_start(
                bqkvt[:].rearrange("p (h m) -> p h m", m=3), bqkv_ap)
            bpt = constp.tile([P, NB], f32)
            nc.scalar.dma_start(bpt[:], bp_ap)
            wp = []
            for fi in range(NB):
                t = constp.tile([P, E], bf16, name=f"wp{fi}", tag=f"wp{fi}")
                nc.scalar.dma_start(t[:], wp_ap[fi * P:(fi + 1) * P, :])
                wp.append(t)
            xts = []
            for eb in range(NB):
                xtile = xtp.tile([P, 1024], fmm, name=f"x{eb}", tag=f"x{eb}")
                nc.sync.dma_start(xtile[:],
                                  xt_ap[eb * P:(eb + 1) * P, :])
                xts.append(xtile)
            oTs = [otp.tile([P, CH], bf16, name=f"oT{hg}", tag=f"oT{hg}")
                   for hg in range(NB)]

            for hg in range(NB):
                # ---------------- QKV for head group hg ----------------
                whg = []
                for eb in range(NB):
                    t = whgp.tile([P, 384], fmm, name=f"w{eb}", tag=f"w{eb}")
                    nc.sync.dma_start(t[:], w3_ap[eb * P:(eb + 1) * P, hg])
                    whg.append(t)
                qT = bigp.tile([P, CH], fmm, tag="qT")
                kT = bigp.tile([P, 1024], f32, tag="kT")
                vT = bigp.tile([P, 1024], f32, tag="vT")
                kdT = bigp.tile([P, 1024], fmm, tag="kdT")
                vn = [vnp.tile([P, 130], fmm, name=f"vn{kb}", tag=f"vn{kb}")
                      for kb in range(8)]
                # Q: local 512 query tokens only (host pre-slid xt so the
                # local half is cols hq0:hq0+512 — see make_in_maps; we use
                # a fixed slice and the host swaps halves for odd cores)
                gps = mmps.tile([P, CH], f32, tag="mm512")
                for eb in range(NB):
                    nc.tensor.matmul(gps[:], whg[eb][:, 0:P],
                                     xts[eb][:, 0:CH],
                                     start=(eb == 0), stop=(eb == NB - 1))
                nc.scalar.activation(
                    qT[:], gps[:], mybir.ActivationFunctionType.Identity,
                    bias=bqkvt[:, hg * 3:hg * 3 + 1])
                for qc in range(2):
                    qs = slice(qc * CH, (qc + 1) * CH)
                    for m, dstT in ((1, kT), (2, vT)):
                        gps = mmps.tile([P, CH], f32, tag="mm512")
                        for eb in range(NB):
                            nc.tensor.matmul(
                                gps[:], whg[eb][:, m * P:(m + 1) * P],
                                xts[eb][:, qs],
                                start=(eb == 0), stop=(eb == NB - 1))
                        nc.vector.tensor_tensor(
                            dstT[:, qs], gps[:],
                            bqkvt[:, hg * 3 + m:hg * 3 + m + 1]
                            .to_broadcast((P, CH)), ADD)

                for kb2 in range(4):
                    k0 = kb2 * 2
                    ks2 = slice(k0 * P, (k0 + 2) * P)
                    psn = smps.tile([P, 2 * P], f32, tag="small")
                    nc.tensor.transpose(psn[:, 0:P], kT[:, k0 * P:(k0 + 1) * P],
                                        identt[:])
                    nc.tensor.transpose(psn[:, P:2 * P],
                                        kT[:, (k0 + 1) * P:(k0 + 2) * P],
                                        identt[:])
                    gmax = wkp.tile([P, 64], f32, tag="gmax")
                    nc.vector.tensor_reduce(
                        gmax[:], psn[:].rearrange("p (g f) -> p g f", f=4),
                        axis=X, op=MAX, apply_absolute_value=True)
                    rg = wkp.tile([P, 64], f32, tag="rg")
                    rgs = wkp.tile([P, 64], f32, tag="rgs")
                    nc.vector.reciprocal_approx_accurate(
                        out=rg[:], in_=gmax[:], scratch=rgs[:])
                    kd = wkp.tile([P, 2 * P], f32, tag="kd")
                    kd_g = kd[:].rearrange("p (g f) -> p g f", f=4)
                    nc.vector.tensor_tensor(
                        kd_g, psn[:].rearrange("p (g f) -> p g f", f=4),
                        rg[:, :, None].to_broadcast((P, 64, 4)), MULT)
                    nc.gpsimd.tensor_scalar(kd[:], kd[:], 1.5, 1.5, MULT, ADD)
                    nc.gpsimd.tensor_scalar(kd[:], kd[:], MAGIC, MAGIC,
                                            ADD, SUB)
                    nc.vector.scalar_tensor_tensor(
                        kd_g, kd_g, 1.5,
                        gmax[:, :, None].to_broadcast((P, 64, 4)), SUB, MULT)
                    ps_t = smps.tile([P, 2 * P], f32, tag="small")
                    nc.tensor.transpose(ps_t[:, 0:P], kd[:, 0:P], identt[:])
                    nc.tensor.transpose(ps_t[:, P:2 * P], kd[:, P:2 * P],
                                        identt[:])
                    nc.vector.tensor_copy(kdT[:, ks2], ps_t[:])
                    psv = smps.tile([P, 2 * P], f32, tag="small")
                    nc.tensor.transpose(psv[:, 0:P], vT[:, k0 * P:(k0 + 1) * P],
                                        identt[:])
                    nc.tensor.transpose(psv[:, P:2 * P],
                                        vT[:, (k0 + 1) * P:(k0 + 2) * P],
                                        identt[:])
                    for j in range(2):
                        kb = k0 + j
                        nc.vector.tensor_copy(vn[kb][:, 0:64],
                                              psv[:, j * P:j * P + 64])
                        nc.vector.tensor_copy(vn[kb][:, 65:129],
                                              psv[:, j * P + 64:(j + 1) * P])
                        nc.scalar.copy(vn[kb][:, 64:65], onescol[:])
                        nc.scalar.copy(vn[kb][:, 129:130], onescol[:])

                # ---------------- attention for head group hg -------------
                for h in range(2):
                    hs = slice(h * 64, (h + 1) * 64)
                    vs = slice(h * 65, (h + 1) * 65)
                    es = []
                    for kb in range(8):
                        ps_s = scps.tile([P, CH], f32, tag="sc")
                        nc.tensor.matmul(
                            ps_s[:], kdT[hs, kb * P:(kb + 1) * P],
                            qT[hs, :], start=True, stop=True)
                        e = esp.tile([P, CH], fmm, name=f"e{kb}",
                                     tag=f"e{kb}")
                        nc.scalar.activation(
                            e[:], ps_s[:], EXP,
                            bias=m8tt[:, kb:kb + 1], scale=1.0 / 12.0)
                        es.append(e)
                    ps_av = avps.tile([65, CH], f32, tag="av")
                    for kb in range(8):
                        nc.tensor.matmul(ps_av[:], vn[kb][:, vs], es[kb][:],
                                         start=(kb == 0), stop=(kb == 7))
                    den = wkp.tile([1, CH], f32, tag="den")
                    nc.scalar.copy(den[:], ps_av[64:65, :])
                    rS = wkp.tile([1, CH], f32, tag="rS")
                    nc.vector.reciprocal_approx_fast(out=rS[:], in_=den[:])
                    rrep = wkp.tile([64, CH], f32, tag="rrep")
                    nc.gpsimd.partition_broadcast(rrep[:], rS[:])
                    nc.vector.tensor_tensor(
                        oTs[hg][hs, :], ps_av[0:64, :], rrep[:], MULT)

            # ---------------- c_proj (fully local) ----------------
            for fo in range(NB):
                pp = mmps.tile([P, CH], f32, tag="mm512")
                for fi in range(NB):
                    nc.tensor.matmul(
                        pp[:], wp[fi][:, fo * P:(fo + 1) * P], oTs[fi][:],
                        start=(fi == 0), stop=(fi == NB - 1))
                yt = wkp.tile([P, CH], f32, tag="yt")
                nc.vector.tensor_tensor(
                    yt[:], pp[:], bpt[:, fo:fo + 1].to_broadcast((P, CH)),
                    ADD)
                nc.sync.dma_start(yt_ap[fo * P:(fo + 1) * P, :], yt[:])

    nc.compile()
    return nc


def make_in_maps(hidden_states, attention_mask, w_attn, b_attn, w_proj, b_proj):
    import ml_dtypes
    x = np.asarray(hidden_states, np.float32).reshape(TOK, E)
    xt = np.ascontiguousarray(x.T)                    # [E, TOK]
    mask = np.asarray(attention_mask, np.float32)     # [4, 1024]
    wa = np.asarray(w_attn, np.float32)
    ba = np.asarray(b_attn, np.float32)
    wpf = np.ascontiguousarray(np.asarray(w_proj, np.float32)
                               .astype(ml_dtypes.bfloat16))
    bp = np.asarray(b_proj, np.float32)

    # wqkv3[e, hg, 0:128]=Wq cols of hg's 2 heads; 128:256=Wk; 256:384=Wv
    wqkv3 = np.empty((E, NB, 384), np.float32)
    bqkv3 = np.empty((P, NB, 3), np.float32)
    for hg in range(NB):
        cs = slice(hg * P, (hg + 1) * P)
        wqkv3[:, hg, 0:128] = wa[:, cs]
        wqkv3[:, hg, 128:256] = wa[:, 1024 + hg * P:1024 + (hg + 1) * P]
        wqkv3[:, hg, 256:384] = wa[:, 2048 + hg * P:2048 + (hg + 1) * P]
        bqkv3[:, hg, 0] = ba[cs]
        bqkv3[:, hg, 1] = ba[1024 + hg * P:1024 + (hg + 1) * P]
        bqkv3[:, hg, 2] = ba[2048 + hg * P:2048 + (hg + 1) * P]
    ident = np.eye(P, dtype=np.float32)
    ones64 = np.ones((1, 64), dtype=np.float32)
    bp_cols = np.ascontiguousarray(bp.reshape(NB, P).T)

    in_maps = []
    for c in range(8):
        bc, hc = c // 2, c % 2
        # roll the token axis so the local query half sits at cols 0:512
        xb = xt[:, bc * 1024:(bc + 1) * 1024]
        if hc == 1:
            xb = np.concatenate([xb[:, 512:], xb[:, :512]], axis=1)
        m8 = (mask[bc] * np.float32(0.125)).reshape(8, 128).T  # [128, 8 kb]
        if hc == 1:
            m8 = np.concatenate([m8[:, 4:], m8[:, :4]], axis=1)
        in_maps.append({
            "xt": np.ascontiguousarray(xb),
            "wqkv3": wqkv3,
            "wp": wpf,
            "bqkv": bqkv3,
            "bp": bp_cols,
            "m8t": np.ascontiguousarray(m8),
            "ident": ident, "ones64": ones64,
            "hcsel": np.array([[hc]], np.int32),
        })
    return in_maps


def kernel(hidden_states, attention_mask, w_attn, b_attn, w_proj, b_proj):
    from concourse import bass_utils
    if "nc" not in _CACHE:
        _CACHE["nc"] = _build()
    nc = _CACHE["nc"]
    in_maps = make_in_maps(hidden_states, attention_mask, w_attn, b_attn,
                           w_proj, b_proj)
    res = bass_utils.run_bass_kernel_spmd(nc, in_maps, core_ids=list(range(8)))
    y = np.empty((TOK, E), dtype=np.float32)
    for c in range(8):
        y[c * CH:(c + 1) * CH, :] = res.results[c]["yt"].T
    return y.reshape(4, 1024, E)
